# revision 54
# baseline (speedup 1.0000x reference)
"""2-layer GAT on 8 Trainium2 NeuronCores.

Strategy: dst-shard nodes across cores; per-edge node-feature access via
dma_gather from a bf16 node table (built on device, AllGathered); segment
softmax + aggregation via one-hot selection matmuls on TensorE.
"""
import numpy as np
import ml_dtypes

import concourse.bacc as bacc
import concourse.bass as bass
import concourse.mybir as mybir
import concourse.tile as tile
from concourse import bass_utils

BF = ml_dtypes.bfloat16
bf16 = mybir.dt.bfloat16
f32 = mybir.dt.float32
i16 = mybir.dt.int16
i32 = mybir.dt.int32

N = 100000
NCORES = 8
SHARD = N // NCORES           # 12500
WIN = 128
W = (SHARD + WIN - 1) // WIN  # 98
SHARD_PAD = W * WIN           # 12544
NQ = 4
QROWS = 2 * SHARD_PAD         # 25088 rows per gather quarter (< 32768)
CHR = SHARD_PAD // NQ         # 3136 table rows contributed per core per chunk
TROW = 128                    # bf16 elems per table row (256B)
FIN = 512
NCLS = 40
AS1_OFF, AD1_OFF = 72, 80
AS2_OFF, AD2_OFF = 48, 56
EPS = 1e-16
AF = mybir.ActivationFunctionType
ALU = mybir.AluOpType

_CACHE = {}


# ---------------------------------------------------------------- host prep
def _prep(edge_index):
    # self loops (PyG default) are handled locally on-chip, not gathered
    src = np.asarray(edge_index[0], np.int64)
    dst = np.asarray(edge_index[1], np.int64)
    # chunk-major table layout: the table is AllGathered in NQ row-chunks of
    # CHR rows per core; chunk q of all cores lands contiguously and equals
    # int16-quarter q of the full table.
    score = src // SHARD
    sl = src % SHARD
    chunk = sl // CHR
    row = chunk * QROWS + score * CHR + (sl % CHR)
    quarter = row // QROWS
    core = dst // SHARD
    dstloc = dst % SHARD
    win = dstloc // WIN
    dstrel = dstloc % WIN

    order = np.lexsort((dstrel, quarter, win, core))
    row_s, q_s, core_s, win_s, rel_s = (
        row[order], quarter[order], core[order], win[order], dstrel[order])

    cell_id = (core_s * W + win_s) * NQ + q_s
    counts = np.bincount(cell_id, minlength=NCORES * W * NQ).reshape(NCORES, W, NQ)
    cellmax = counts.max(axis=0)
    cellmax16 = ((cellmax + 15) // 16) * 16
    kq = (cellmax16 + 127) // 128
    cell_slots = kq * 128
    cell_off = np.zeros((W, NQ), np.int64)
    cell_off[:, 1:] = np.cumsum(cell_slots, axis=1)[:, :-1]
    win_slots = cell_slots.sum(axis=1)
    win_off = np.zeros(W, np.int64)
    win_off[1:] = np.cumsum(win_slots)[:-1]
    total_slots = int(win_slots.sum())
    K = win_slots // 128

    idx16 = np.zeros((NCORES, total_slots), np.int16)
    relv = np.full((NCORES, total_slots), -1.0, np.float32)
    starts = np.zeros(NCORES * W * NQ + 1, np.int64)
    np.cumsum(np.bincount(cell_id, minlength=NCORES * W * NQ), out=starts[1:])
    for c in range(NCORES):
        for w in range(W):
            base = win_off[w]
            for q in range(NQ):
                cid = (c * W + w) * NQ + q
                s0, s1 = starts[cid], starts[cid + 1]
                n = s1 - s0
                o = base + cell_off[w, q]
                idx16[c, o:o + n] = (row_s[s0:s1] - q * QROWS).astype(np.int16)
                relv[c, o:o + n] = rel_s[s0:s1].astype(np.float32)
                m16 = cellmax16[w, q]
                idx16[c, o + n:o + m16] = 0
                idx16[c, o + m16:o + cell_slots[w, q]] = -1
    # transposed one-hot selection matrices, streamed per window:
    # seltd[c][j, s] = 1.0 where slot s of core c targets dst-rel j
    jj = np.arange(WIN, dtype=np.float32)[:, None]
    seltd = [(relv[c][None, :] == jj).astype(BF) for c in range(NCORES)]
    return {
        "idx16": idx16, "dstrel": relv, "cellmax16": cellmax16, "kq": kq,
        "cell_off": cell_off, "win_off": win_off, "win_slots": win_slots,
        "K": K, "total_slots": total_slots,
        "counts": counts, "cellmin": counts.min(axis=0), "seltd": seltd,
    }


def _build_inputs(meta, inputs):
    x = np.asarray(inputs["x"], np.float32)
    W1 = np.asarray(inputs["W1"], np.float32)
    W2 = np.asarray(inputs["W2"], np.float32)
    as1 = np.asarray(inputs["att_src1"], np.float32).reshape(8, 8)
    ad1 = np.asarray(inputs["att_dst1"], np.float32).reshape(8, 8)
    as2 = np.asarray(inputs["att_src2"], np.float32).reshape(NCLS)
    ad2 = np.asarray(inputs["att_dst2"], np.float32).reshape(NCLS)
    b1 = np.asarray(inputs["b1"], np.float32)
    b2 = np.asarray(inputs["b2"], np.float32)

    attsd = np.zeros((64, 16), np.float32)
    for h in range(8):
        attsd[h * 8:(h + 1) * 8, h] = as1[h]
        attsd[h * 8:(h + 1) * 8, 8 + h] = ad1[h]
    att2sd = np.stack([as2, ad2], axis=1)

    common = {
        "w1": W1.astype(BF),
        "w2": W2.astype(BF),
        "attsd": attsd.astype(BF),
        "att2sd": att2sd.astype(BF),
        "b1c": np.tile(b1[None, :], (128, 1)).astype(np.float32),
        "b2c": np.tile(b2[None, :], (128, 1)).astype(np.float32),
        "rconst": np.tile(np.arange(128, dtype=np.float32)[None, :],
                          (128, 1)).astype(BF),
        "ident": np.eye(128, dtype=np.float32).astype(BF),
        "ident32": np.eye(128, dtype=np.float32),
        "zeros": np.zeros((128, 8 * TROW), BF),
    }
    S = int(meta["total_slots"])
    maps = []
    for core in range(NCORES):
        idx = meta["idx16"][core]
        idx_in = np.tile(idx.reshape(S // 16, 16).T, (8, 1))
        drel_in = meta["dstrel"][core].reshape(S // 128, 128).T.astype(BF)
        m = dict(common)
        m["xT"] = np.ascontiguousarray(
            x[core * SHARD:(core + 1) * SHARD].T).astype(BF)
        m["idxs"] = np.ascontiguousarray(idx_in)
        m["drel"] = np.ascontiguousarray(drel_in)
        m["ccnt"] = np.ascontiguousarray(
            meta["counts"][core].reshape(1, W * NQ).astype(np.int32))
        m["seltd"] = meta["seltd"][core]
        maps.append(m)
    return maps


# ---------------------------------------------------------------- bass build
def _edge_phase(nc, tc, meta, tbl_qs, idx_s, drel_s, rconst_s, ident_s,
                adw, acc, maxK, layer, zeros_d, ccnt_s, cregs, seltd):
    kq, cellmax16 = meta["kq"], meta["cellmax16"]
    cell_off, win_off, K = meta["cell_off"], meta["win_off"], meta["K"]
    cellmin = meta["cellmin"]
    H = 8 if layer == 1 else 1
    VPW = 72 if layer == 1 else 41
    GW = 9 if layer == 1 else 41
    AOFF = AS1_OFF if layer == 1 else AS2_OFF
    with tc.tile_pool(name=f"eg{layer}", bufs=4) as gpool, \
         tc.tile_pool(name=f"ep{layer}", bufs=2) as pool, \
         tc.tile_pool(name=f"epa{layer}", bufs=2, space="PSUM") as psa, \
         tc.tile_pool(name=f"epd{layer}", bufs=2, space="PSUM") as psd:
        for w in range(W):
            kw = int(K[w])
            adww = adw[:].rearrange("p (w h) -> p w h", w=W)[:, w, 0:H] \
                if layer == 1 else adw[:, w:w + 1]
            g = gpool.tile([128, maxK, TROW], bf16, tag="gather")
            if cregs is not None:
                nc.gpsimd.reg_load(cregs, ccnt_s[0:1, w * NQ:(w + 1) * NQ])
            for q in range(NQ):
                nidx = int(kq[w, q]) * 128
                if nidx == 0:
                    continue
                off_blk = int(cell_off[w, q]) // 128
                slot0 = int(win_off[w]) + int(cell_off[w, q])
                cmn = int(cellmin[w, q])
                if cmn < nidx:
                    zb = cmn // 128
                    nc.sync.dma_start(
                        g[:, off_blk + zb:off_blk + nidx // 128, :],
                        zeros_d[:, 0:(nidx // 128 - zb) * TROW].rearrange(
                            "p (n t) -> p n t", t=TROW))
                nc.gpsimd.dma_gather(
                    out_ap=g[:, off_blk:off_blk + nidx // 128, :],
                    in_ap=tbl_qs[q][:],
                    idxs_ap=idx_s[:, slot0 // 16:(slot0 + nidx) // 16],
                    num_idxs=nidx,
                    num_idxs_reg=(cregs[q] if cregs is not None
                                  else int(cellmax16[w, q])),
                    elem_size=TROW,
                    single_packet=False,
                    queue_num=q)
            sel = pool.tile([128, maxK * 128], bf16, tag="sel")
            nc.vector.tensor_tensor(
                out=sel[:, 0:kw * 128].rearrange("p (k j) -> p k j", k=kw),
                in0=drel_s[:, int(win_off[w]) // 128:int(win_off[w]) // 128 + kw]
                    .rearrange("p (k x) -> p k x", x=1).to_broadcast([128, kw, 128]),
                in1=rconst_s[:].rearrange("p (x j) -> p x j", x=1)
                    .to_broadcast([128, kw, 128]),
                op=ALU.is_equal)
            selt = pool.tile([128, maxK * 128], bf16, tag="selt")
            nc.sync.dma_start(
                selt[:, 0:kw * 128],
                seltd[:, int(win_off[w]):int(win_off[w]) + kw * 128])
            pad_ = psd.tile([128, maxK * H], f32, tag="adst")
            for k in range(kw):
                nc.tensor.matmul(
                    pad_[:, k * H:(k + 1) * H],
                    lhsT=selt[:, k * 128:(k + 1) * 128],
                    rhs=adww, start=True, stop=True)
            ep = pool.tile([128, maxK * H], f32, tag="ep")
            nc.vector.tensor_tensor(
                out=ep[:, 0:kw * H].rearrange("p (k h) -> p k h", k=kw),
                in0=g[:, 0:kw, AOFF:AOFF + H],
                in1=pad_[:, 0:kw * H].rearrange("p (k h) -> p k h", k=kw),
                op=ALU.add)
            pex1 = pool.tile([128, maxK * H], bf16, tag="pex1")
            nc.scalar.activation(pex1[:, 0:kw * H], ep[:, 0:kw * H], AF.Exp)
            pex = pool.tile([128, maxK * H], bf16, tag="pex")
            nc.scalar.activation(pex[:, 0:kw * H], ep[:, 0:kw * H], AF.Exp,
                                 scale=0.2)
            nc.vector.tensor_tensor(out=pex[:, 0:kw * H],
                                    in0=pex[:, 0:kw * H],
                                    in1=pex1[:, 0:kw * H], op=ALU.max)
            vp = pool.tile([128, maxK, VPW], bf16, tag="vp")
            nc.vector.tensor_tensor(
                out=vp[:, 0:kw, :].rearrange("p k (h x) -> p k h x", h=H),
                in0=g[:, 0:kw, 0:VPW].rearrange("p k (h x) -> p k h x", h=H),
                in1=pex[:, 0:kw * H].rearrange("p (k h x) -> p k h x", k=kw, x=1)
                    .to_broadcast([128, kw, H, GW]),
                op=ALU.mult)
            pagg = psa.tile([128, VPW], f32, tag="agg")
            for k in range(kw):
                nc.tensor.matmul(pagg[:], lhsT=sel[:, k * 128:(k + 1) * 128],
                                 rhs=vp[:, k, :], start=(k == 0),
                                 stop=(k == kw - 1))
            nc.scalar.activation(acc[:, w * VPW:(w + 1) * VPW], pagg[:],
                                 AF.Copy)


def _build(meta):
    kq = meta["kq"]; cellmax16 = meta["cellmax16"]
    K = meta["K"]; S = int(meta["total_slots"])
    SB = S // 128
    maxK = int(K.max())
    KT = FIN // 128

    nc = bacc.Bacc("TRN2", target_bir_lowering=False, debug=False,
                   num_devices=NCORES, num_swdge_queues=4)
    xT = nc.dram_tensor("xT", [FIN, SHARD], bf16, kind="ExternalInput")
    w1 = nc.dram_tensor("w1", [FIN, 64], bf16, kind="ExternalInput")
    w2 = nc.dram_tensor("w2", [64, NCLS], bf16, kind="ExternalInput")
    attsd = nc.dram_tensor("attsd", [64, 16], bf16, kind="ExternalInput")
    att2sd = nc.dram_tensor("att2sd", [NCLS, 2], bf16, kind="ExternalInput")
    b1c = nc.dram_tensor("b1c", [128, 64], f32, kind="ExternalInput")
    b2c = nc.dram_tensor("b2c", [128, NCLS], f32, kind="ExternalInput")
    rconst = nc.dram_tensor("rconst", [128, 128], bf16, kind="ExternalInput")
    ident = nc.dram_tensor("ident", [128, 128], bf16, kind="ExternalInput")
    ident32 = nc.dram_tensor("ident32", [128, 128], f32, kind="ExternalInput")
    idxs = nc.dram_tensor("idxs", [128, S // 16], i16, kind="ExternalInput")
    drel = nc.dram_tensor("drel", [128, SB], bf16, kind="ExternalInput")
    seltd = nc.dram_tensor("seltd", [128, S], bf16, kind="ExternalInput")
    zeros = nc.dram_tensor("zeros", [128, 8 * TROW], bf16, kind="ExternalInput")
    ccnt = nc.dram_tensor("ccnt", [1, W * NQ], i32, kind="ExternalInput")
    out = nc.dram_tensor("out", [SHARD_PAD, NCLS], f32, kind="ExternalOutput")

    with tile.TileContext(nc) as tc:
        with tc.tile_pool(name="dram", bufs=1, space="DRAM") as dpool, \
             tc.tile_pool(name="persist", bufs=1) as pp:
            tbl_shard = dpool.tile([SHARD_PAD, TROW], bf16)
            tbl_qs = [dpool.tile([QROWS, TROW], bf16, name=f"tbl_q{q}")
                      for q in range(NQ)]
            tbl_qs2 = [dpool.tile([QROWS, TROW], bf16, name=f"tbl2_q{q}")
                       for q in range(NQ)]

            ident_s = pp.tile([128, 128], bf16)
            nc.sync.dma_start(ident_s[:], ident[:])
            ident32_s = pp.tile([128, 128], f32)
            nc.sync.dma_start(ident32_s[:], ident32[:])
            rconst_s = pp.tile([128, 128], bf16)
            nc.sync.dma_start(rconst_s[:], rconst[:])
            drel_s = pp.tile([128, SB], bf16)
            nc.sync.dma_start(drel_s[:], drel[:])
            idx_s = pp.tile([128, S // 16], i16)
            nc.sync.dma_start(idx_s[:], idxs[:])
            b1_s = pp.tile([128, 64], f32)
            nc.sync.dma_start(b1_s[:], b1c[:])
            b2_s = pp.tile([128, NCLS], f32)
            nc.sync.dma_start(b2_s[:], b2c[:])
            ccnt_s = pp.tile([1, W * NQ], i32)
            nc.sync.dma_start(ccnt_s[:], ccnt[:])
            cregs = None  # register num_idxs_reg: suspected runtime crash
            acc = pp.tile([128, W * 72], f32, tag="acc")
            adw = pp.tile([128, W * 8], bf16, tag="adw")
            asw = pp.tile([128, W * 8], bf16, tag="asw")
            hloc = pp.tile([128, W * 72], bf16, tag="hloc")
            h2 = pp.tile([128, W * 64], f32, tag="h2")
            h2t = pp.tile([64, SHARD_PAD], bf16, tag="h2t")

            # phase A
            with tc.tile_pool(name="pa", bufs=2) as pool, \
                 tc.tile_pool(name="pac", bufs=1) as cpool, \
                 tc.tile_pool(name="pap", bufs=2, space="PSUM") as psum:
                w1_s = cpool.tile([128, KT, 64], bf16)
                nc.sync.dma_start(w1_s[:], w1[:].rearrange("(k p) m -> p k m", p=128))
                att_s = cpool.tile([64, 16], bf16)
                nc.sync.dma_start(att_s[:], attsd[:])
                hT = cpool.tile([64, SHARD], bf16)
                aT = cpool.tile([16, SHARD], bf16)
                stgbufs = [cpool.tile([128, TROW], bf16, name=f"stgA{i}")
                           for i in range(2)]
                for s_ in stgbufs:
                    nc.vector.memset(s_[:], 0.0)
                    nc.vector.memset(
                        s_[:, 0:72].rearrange("p (h c) -> p h c", h=8)[:, :, 8],
                        1.0)

                def emit_stg(w):
                    stg = stgbufs[w % 2]
                    n0 = w * 128
                    nw = min(128, SHARD - n0)
                    pt = psum.tile([128, 64], bf16, tag="tp")
                    nc.tensor.transpose(pt[0:nw, 0:64], hT[:, n0:n0 + nw],
                                        ident_s[0:64, 0:64])
                    nc.scalar.activation(
                        stg[0:nw, 0:72].rearrange("p (h c) -> p h c", h=8)[:, :, 0:8],
                        pt[0:nw, 0:64].rearrange("p (h c) -> p h c", h=8),
                        AF.Copy)
                    pt2 = psum.tile([128, 16], bf16, tag="tp2")
                    nc.tensor.transpose(pt2[0:nw, :], aT[:, n0:n0 + nw],
                                        ident_s[0:16, 0:16])
                    nc.scalar.activation(stg[0:nw, AS1_OFF:AS1_OFF + 16],
                                         pt2[0:nw, :], AF.Copy)
                    nc.sync.dma_start(
                        tbl_shard[:].rearrange("(w p) t -> p w t", p=128)[:, w, :],
                        stg[:])

                CH = 500
                next_w = 0
                for ci in range(SHARD // CH):
                    s0, s1 = ci * CH, (ci + 1) * CH
                    xt_t = pool.tile([128, KT, CH], bf16, tag="xt")
                    nc.sync.dma_start(
                        xt_t[:], xT[:, s0:s1].rearrange("(k p) n -> p k n", p=128))
                    ps = psum.tile([64, CH], f32, tag="hps")
                    for k in range(KT):
                        nc.tensor.matmul(ps[:], lhsT=w1_s[:, k, :],
                                         rhs=xt_t[:, k, :],
                                         start=(k == 0), stop=(k == KT - 1))
                    nc.vector.tensor_copy(hT[:, s0:s1], ps[:])
                    ps2 = psum.tile([16, CH], f32, tag="aps")
                    nc.tensor.matmul(ps2[:], lhsT=att_s[:], rhs=hT[:, s0:s1],
                                     start=True, stop=True)
                    nc.vector.tensor_copy(aT[:, s0:s1], ps2[:])
                    # emit table rows for fully-computed windows right away so
                    # the chunked AllGathers can start while hT is still going
                    while next_w < W and (next_w + 1) * 128 <= s1:
                        emit_stg(next_w)
                        next_w += 1
                while next_w < W:
                    emit_stg(next_w)
                    next_w += 1

            for q in range(NQ):
                nc.gpsimd.collective_compute(
                    "AllGather", ALU.bypass,
                    replica_groups=[list(range(NCORES))],
                    ins=[tbl_shard[q * CHR:(q + 1) * CHR, :].opt()],
                    outs=[tbl_qs[q][:].opt()])
            nc.sync.dma_start(
                adw[:].rearrange("p (w h) -> p w h", w=W),
                tbl_shard[:].rearrange("(w p) t -> p w t", p=128)
                [:, :, AD1_OFF:AD1_OFF + 8])
            nc.sync.dma_start(
                asw[:].rearrange("p (w h) -> p w h", w=W),
                tbl_shard[:].rearrange("(w p) t -> p w t", p=128)
                [:, :, AS1_OFF:AS1_OFF + 8])
            nc.sync.dma_start(
                hloc[:].rearrange("p (w c) -> p w c", w=W),
                tbl_shard[:].rearrange("(w p) t -> p w t", p=128)[:, :, 0:72])

            _edge_phase(nc, tc, meta, tbl_qs, idx_s, drel_s, rconst_s,
                        ident_s, adw, acc, maxK, layer=1,
                        zeros_d=zeros, ccnt_s=ccnt_s, cregs=cregs,
                        seltd=seltd)

            # self-loop add + f1 normalization + phase D, chunked over window
            # halves so the first half overlaps the tail of edge phase 1 and
            # the layer-2 collectives start earlier
            with tc.tile_pool(name="f1", bufs=1) as pool, \
                 tc.tile_pool(name="pdc", bufs=1) as cpool, \
                 tc.tile_pool(name="pdp", bufs=1, space="PSUM") as psum:
                w2_s = cpool.tile([64, NCLS], bf16)
                nc.sync.dma_start(w2_s[:], w2[:])
                att2_s = cpool.tile([NCLS, 2], bf16)
                nc.sync.dma_start(att2_s[:], att2sd[:])
                stgbufs2 = [cpool.tile([128, TROW], bf16, name=f"stgD{i}")
                            for i in range(2)]
                for s_ in stgbufs2:
                    nc.vector.memset(s_[:], 0.0)
                    nc.vector.memset(s_[:, NCLS:NCLS + 1], 1.0)
                for w0, w1 in ((0, W // 2), (W // 2, W)):
                    nw = w1 - w0
                    gT = pool.tile([NCLS, (W - W // 2) * 128], bf16, tag="gT")
                    a2T = pool.tile([2, (W - W // 2) * 128], bf16, tag="a2T")
                    # self-loop contribution (computed locally, never gathered)
                    z = pool.tile([128, (W - W // 2) * 8], f32, tag="z")
                    nc.vector.tensor_tensor(out=z[:, 0:nw * 8],
                                            in0=asw[:, w0 * 8:w1 * 8],
                                            in1=adw[:, w0 * 8:w1 * 8],
                                            op=ALU.add)
                    e1 = pool.tile([128, (W - W // 2) * 8], bf16, tag="e1")
                    nc.scalar.activation(e1[:, 0:nw * 8], z[:, 0:nw * 8],
                                         AF.Exp)
                    pexs = pool.tile([128, (W - W // 2) * 8], bf16, tag="pexs")
                    nc.scalar.activation(pexs[:, 0:nw * 8], z[:, 0:nw * 8],
                                         AF.Exp, scale=0.2)
                    nc.vector.tensor_tensor(out=pexs[:, 0:nw * 8],
                                            in0=pexs[:, 0:nw * 8],
                                            in1=e1[:, 0:nw * 8], op=ALU.max)
                    accs = acc[:, w0 * 72:w1 * 72]
                    vps = pool.tile([128, (W - W // 2) * 72], bf16, tag="vps")
                    nc.vector.tensor_tensor(
                        out=vps[:, 0:nw * 72]
                            .rearrange("p (w h x) -> p w h x", w=nw, h=8),
                        in0=hloc[:, w0 * 72:w1 * 72]
                            .rearrange("p (w h x) -> p w h x", w=nw, h=8),
                        in1=pexs[:, 0:nw * 8]
                            .rearrange("p (w h x) -> p w h x", w=nw, x=1)
                            .to_broadcast([128, nw, 8, 9]),
                        op=ALU.mult)
                    nc.vector.tensor_tensor(out=accs, in0=accs,
                                            in1=vps[:, 0:nw * 72],
                                            op=ALU.add)
                    # f1: normalize + bias + ELU
                    accv = accs.rearrange("p (w h x) -> p w h x", w=nw, h=8)
                    den = pool.tile([128, (W - W // 2) * 8], f32, tag="den")
                    dv = den[:, 0:nw * 8].rearrange("p (w h) -> p w h", w=nw)
                    nc.vector.tensor_scalar_add(dv, accv[:, :, :, 8], EPS)
                    nc.vector.reciprocal(den[:, 0:nw * 8], den[:, 0:nw * 8])
                    h2s = h2[:, w0 * 64:w1 * 64]
                    h2v = h2s.rearrange("p (w h x) -> p w h x", w=nw, h=8)
                    nc.vector.tensor_tensor(
                        out=h2v, in0=accv[:, :, :, 0:8],
                        in1=den[:, 0:nw * 8]
                            .rearrange("p (w h x) -> p w h x", w=nw, x=1)
                            .to_broadcast([128, nw, 8, 8]),
                        op=ALU.mult)
                    nc.vector.tensor_tensor(
                        out=h2s.rearrange("p (w x) -> p w x", w=nw),
                        in0=h2s.rearrange("p (w x) -> p w x", w=nw),
                        in1=b1_s[:].rearrange("p (o x) -> p o x", o=1)
                            .to_broadcast([128, nw, 64]),
                        op=ALU.add)
                    t2 = pool.tile([128, (W - W // 2) * 64], bf16, tag="t2")
                    t2s = t2[:, 0:nw * 64]
                    nc.vector.tensor_scalar_min(t2s, h2s, 0.0)
                    nc.scalar.activation(t2s, t2s, AF.Exp)
                    nc.vector.tensor_scalar_add(t2s, t2s, -1.0)
                    nc.vector.tensor_scalar_min(t2s, t2s, 0.0)
                    nc.vector.tensor_scalar_max(h2s, h2s, 0.0)
                    nc.vector.tensor_tensor(out=h2s, in0=h2s, in1=t2s,
                                            op=ALU.add)
                    # phase D for this window range
                    for w in range(w0, w1):
                        pt = psum.tile([64, 128], f32, tag="t")
                        nc.tensor.transpose(pt[:], h2[:, w * 64:(w + 1) * 64],
                                            ident32_s[:])
                        nc.scalar.activation(h2t[:, w * 128:(w + 1) * 128],
                                             pt[:], AF.Copy)
                    CH2 = 512
                    s0 = w0 * 128
                    while s0 < w1 * 128:
                        s1 = min(s0 + CH2, w1 * 128)
                        l0, l1 = s0 - w0 * 128, s1 - w0 * 128
                        ps = psum.tile([NCLS, CH2], f32, tag="g")
                        nc.tensor.matmul(ps[:, 0:s1 - s0], lhsT=w2_s[:],
                                         rhs=h2t[:, s0:s1], start=True,
                                         stop=True)
                        nc.vector.tensor_copy(gT[:, l0:l1], ps[:, 0:s1 - s0])
                        ps2 = psum.tile([2, CH2], f32, tag="a2")
                        nc.tensor.matmul(ps2[:, 0:s1 - s0], lhsT=att2_s[:],
                                         rhs=gT[:, l0:l1], start=True,
                                         stop=True)
                        nc.vector.tensor_copy(a2T[:, l0:l1], ps2[:, 0:s1 - s0])
                        s0 = s1
                    for w in range(w0, w1):
                        lw = w - w0
                        stg = stgbufs2[w % 2]
                        pt = psum.tile([128, 64], bf16, tag="tg")
                        nc.tensor.transpose(pt[:, 0:NCLS],
                                            gT[:, lw * 128:(lw + 1) * 128],
                                            ident_s[0:NCLS, 0:NCLS])
                        nc.scalar.activation(stg[:, 0:NCLS], pt[:, 0:NCLS],
                                             AF.Copy)
                        pt2 = psum.tile([128, 2], bf16, tag="ta2")
                        nc.tensor.transpose(pt2[:],
                                            a2T[:, lw * 128:(lw + 1) * 128],
                                            ident_s[0:2, 0:2])
                        nc.scalar.activation(
                            stg[:, AS2_OFF:AD2_OFF + 8].rearrange(
                                "p (a x) -> p a x", a=2)[:, :, 0],
                            pt2[:], AF.Copy)
                        nc.sync.dma_start(
                            tbl_shard[:].rearrange("(w p) t -> p w t",
                                                   p=128)[:, w, :],
                            stg[:])

            for q in range(NQ):
                nc.gpsimd.collective_compute(
                    "AllGather", ALU.bypass,
                    replica_groups=[list(range(NCORES))],
                    ins=[tbl_shard[q * CHR:(q + 1) * CHR, :].opt()],
                    outs=[tbl_qs2[q][:].opt()])
            nc.sync.dma_start(
                adw[:, 0:W].rearrange("p (w h) -> p w h", w=W),
                tbl_shard[:].rearrange("(w p) t -> p w t", p=128)
                [:, :, AD2_OFF:AD2_OFF + 1])
            nc.sync.dma_start(
                asw[:, 0:W].rearrange("p (w h) -> p w h", w=W),
                tbl_shard[:].rearrange("(w p) t -> p w t", p=128)
                [:, :, AS2_OFF:AS2_OFF + 1])
            nc.sync.dma_start(
                hloc[:, 0:W * 41].rearrange("p (w c) -> p w c", w=W),
                tbl_shard[:].rearrange("(w p) t -> p w t", p=128)[:, :, 0:41])

            acc2 = pp.tile([128, W * 41], f32, tag="acc")
            _edge_phase(nc, tc, meta, tbl_qs2, idx_s, drel_s, rconst_s,
                        ident_s, adw, acc2, maxK, layer=2,
                        zeros_d=zeros, ccnt_s=ccnt_s, cregs=cregs,
                        seltd=seltd)

            with tc.tile_pool(name="f2", bufs=2) as pool:
                for w0, w1 in ((0, W // 2), (W // 2, W)):
                    nw = w1 - w0
                    z = pool.tile([128, W - W // 2], f32, tag="z2")
                    nc.vector.tensor_tensor(out=z[:, 0:nw], in0=asw[:, w0:w1],
                                            in1=adw[:, w0:w1], op=ALU.add)
                    e1 = pool.tile([128, W - W // 2], bf16, tag="e12")
                    nc.scalar.activation(e1[:, 0:nw], z[:, 0:nw], AF.Exp)
                    pexs = pool.tile([128, W - W // 2], bf16, tag="pexs2")
                    nc.scalar.activation(pexs[:, 0:nw], z[:, 0:nw], AF.Exp,
                                         scale=0.2)
                    nc.vector.tensor_tensor(out=pexs[:, 0:nw],
                                            in0=pexs[:, 0:nw],
                                            in1=e1[:, 0:nw], op=ALU.max)
                    acc2s = acc2[:, w0 * 41:w1 * 41]
                    vps = pool.tile([128, (W - W // 2) * 41], f32, tag="vps2")
                    nc.vector.tensor_tensor(
                        out=vps[:, 0:nw * 41]
                            .rearrange("p (w x) -> p w x", w=nw),
                        in0=hloc[:, w0 * 41:w1 * 41]
                            .rearrange("p (w x) -> p w x", w=nw),
                        in1=pexs[:, 0:nw].rearrange("p (w x) -> p w x", x=1)
                            .to_broadcast([128, nw, 41]),
                        op=ALU.mult)
                    nc.vector.tensor_tensor(out=acc2s, in0=acc2s,
                                            in1=vps[:, 0:nw * 41], op=ALU.add)
                    accv = acc2s.rearrange("p (w x) -> p w x", w=nw)
                    den = pool.tile([128, W - W // 2], f32, tag="den2")
                    nc.vector.tensor_scalar_add(den[:, 0:nw], accv[:, :, 40],
                                                EPS)
                    nc.vector.reciprocal(den[:, 0:nw], den[:, 0:nw])
                    o = pool.tile([128, (W - W // 2) * NCLS], f32, tag="o2")
                    ov = o[:, 0:nw * NCLS].rearrange("p (w x) -> p w x", w=nw)
                    nc.vector.tensor_tensor(
                        out=ov, in0=accv[:, :, 0:NCLS],
                        in1=den[:, 0:nw].rearrange("p (w x) -> p w x", x=1)
                            .to_broadcast([128, nw, NCLS]),
                        op=ALU.mult)
                    nc.vector.tensor_tensor(
                        out=ov, in0=ov,
                        in1=b2_s[:].rearrange("p (o x) -> p o x", o=1)
                            .to_broadcast([128, nw, NCLS]),
                        op=ALU.add)
                    mx = pool.tile([128, W - W // 2], f32, tag="mx2")
                    nc.vector.tensor_reduce(out=mx[:, 0:nw], in_=ov,
                                            op=ALU.max,
                                            axis=mybir.AxisListType.X)
                    nc.vector.tensor_tensor(
                        out=ov, in0=ov,
                        in1=mx[:, 0:nw].rearrange("p (w x) -> p w x", x=1)
                            .to_broadcast([128, nw, NCLS]),
                        op=ALU.subtract)
                    nc.scalar.activation(o[:, 0:nw * NCLS],
                                         o[:, 0:nw * NCLS], AF.Exp)
                    sm = pool.tile([128, W - W // 2], f32, tag="sm2")
                    nc.vector.tensor_reduce(out=sm[:, 0:nw], in_=ov,
                                            op=ALU.add,
                                            axis=mybir.AxisListType.X)
                    nc.vector.reciprocal(sm[:, 0:nw], sm[:, 0:nw])
                    nc.vector.tensor_tensor(
                        out=ov, in0=ov,
                        in1=sm[:, 0:nw].rearrange("p (w x) -> p w x", x=1)
                            .to_broadcast([128, nw, NCLS]),
                        op=ALU.mult)
                    nc.sync.dma_start(
                        out[:].rearrange("(w p) x -> p w x", p=128)[:, w0:w1],
                        ov)
    nc.finalize()
    return nc


# ---------------------------------------------------------------- entry point
def kernel(**inputs):
    edge = np.asarray(inputs["edge_index"])
    key = hash(edge[:, :1024].tobytes()) ^ hash(edge.shape)
    if key not in _CACHE:
        meta = _prep(edge)
        nc = _build(meta)
        _CACHE[key] = (meta, nc)
    meta, nc = _CACHE[key]
    maps = _build_inputs(meta, inputs)
    res = bass_utils.run_bass_kernel_spmd(
        nc, maps, core_ids=list(range(NCORES)), trace=False)
    out = np.zeros((N, NCLS), np.float32)
    for core in range(NCORES):
        o = np.asarray(res.results[core]["out"]).reshape(SHARD_PAD, NCLS)
        out[core * SHARD:(core + 1) * SHARD] = o[:SHARD]
    return out



# revision 57
# speedup vs baseline: 1.0243x; 1.0243x over previous
"""2-layer GAT on 8 Trainium2 NeuronCores.

Strategy: dst-shard nodes across cores; per-edge node-feature access via
dma_gather from a bf16 node table (built on device, AllGathered); segment
softmax + aggregation via one-hot selection matmuls on TensorE.
"""
import numpy as np
import ml_dtypes

import concourse.bacc as bacc
import concourse.bass as bass
import concourse.mybir as mybir
import concourse.tile as tile
from concourse import bass_utils

BF = ml_dtypes.bfloat16
bf16 = mybir.dt.bfloat16
f32 = mybir.dt.float32
i16 = mybir.dt.int16
i32 = mybir.dt.int32

N = 100000
NCORES = 8
SHARD = N // NCORES           # 12500
WIN = 128
W = (SHARD + WIN - 1) // WIN  # 98
SHARD_PAD = W * WIN           # 12544
NQ = 4
QROWS = 2 * SHARD_PAD         # 25088 rows per gather quarter (< 32768)
CHR = SHARD_PAD // NQ         # 3136 table rows contributed per core per chunk
TROW = 128                    # bf16 elems per table row (256B)
FIN = 512
NCLS = 40
AS1_OFF, AD1_OFF = 72, 80
AS2_OFF, AD2_OFF = 48, 56
EPS = 1e-16
AF = mybir.ActivationFunctionType
ALU = mybir.AluOpType

_CACHE = {}


# ---------------------------------------------------------------- host prep
def _prep(edge_index):
    # self loops (PyG default) are handled locally on-chip, not gathered
    src = np.asarray(edge_index[0], np.int64)
    dst = np.asarray(edge_index[1], np.int64)
    # chunk-major table layout: the table is AllGathered in NQ row-chunks of
    # CHR rows per core; chunk q of all cores lands contiguously and equals
    # int16-quarter q of the full table.
    score = src // SHARD
    sl = src % SHARD
    chunk = sl // CHR
    row = chunk * QROWS + score * CHR + (sl % CHR)
    quarter = row // QROWS
    core = dst // SHARD
    dstloc = dst % SHARD
    win = dstloc // WIN
    dstrel = dstloc % WIN

    order = np.lexsort((dstrel, quarter, win, core))
    row_s, q_s, core_s, win_s, rel_s = (
        row[order], quarter[order], core[order], win[order], dstrel[order])

    cell_id = (core_s * W + win_s) * NQ + q_s
    counts = np.bincount(cell_id, minlength=NCORES * W * NQ).reshape(NCORES, W, NQ)
    cellmax = counts.max(axis=0)
    cellmax16 = ((cellmax + 15) // 16) * 16
    kq = (cellmax16 + 127) // 128
    cell_slots = kq * 128
    cell_off = np.zeros((W, NQ), np.int64)
    cell_off[:, 1:] = np.cumsum(cell_slots, axis=1)[:, :-1]
    win_slots = cell_slots.sum(axis=1)
    win_off = np.zeros(W, np.int64)
    win_off[1:] = np.cumsum(win_slots)[:-1]
    total_slots = int(win_slots.sum())
    K = win_slots // 128

    idx16 = np.zeros((NCORES, total_slots), np.int16)
    relv = np.full((NCORES, total_slots), -1.0, np.float32)
    starts = np.zeros(NCORES * W * NQ + 1, np.int64)
    np.cumsum(np.bincount(cell_id, minlength=NCORES * W * NQ), out=starts[1:])
    for c in range(NCORES):
        for w in range(W):
            base = win_off[w]
            for q in range(NQ):
                cid = (c * W + w) * NQ + q
                s0, s1 = starts[cid], starts[cid + 1]
                n = s1 - s0
                o = base + cell_off[w, q]
                idx16[c, o:o + n] = (row_s[s0:s1] - q * QROWS).astype(np.int16)
                relv[c, o:o + n] = rel_s[s0:s1].astype(np.float32)
                m16 = cellmax16[w, q]
                idx16[c, o + n:o + m16] = 0
                idx16[c, o + m16:o + cell_slots[w, q]] = -1
    # transposed one-hot selection matrices, streamed per window:
    # seltd[c][j, s] = 1.0 where slot s of core c targets dst-rel j
    jj = np.arange(WIN, dtype=np.float32)[:, None]
    seltd = [(relv[c][None, :] == jj).astype(BF) for c in range(NCORES)]
    return {
        "idx16": idx16, "dstrel": relv, "cellmax16": cellmax16, "kq": kq,
        "cell_off": cell_off, "win_off": win_off, "win_slots": win_slots,
        "K": K, "total_slots": total_slots,
        "counts": counts, "cellmin": counts.min(axis=0), "seltd": seltd,
    }


def _build_inputs(meta, inputs):
    x = np.asarray(inputs["x"], np.float32)
    W1 = np.asarray(inputs["W1"], np.float32)
    W2 = np.asarray(inputs["W2"], np.float32)
    as1 = np.asarray(inputs["att_src1"], np.float32).reshape(8, 8)
    ad1 = np.asarray(inputs["att_dst1"], np.float32).reshape(8, 8)
    as2 = np.asarray(inputs["att_src2"], np.float32).reshape(NCLS)
    ad2 = np.asarray(inputs["att_dst2"], np.float32).reshape(NCLS)
    b1 = np.asarray(inputs["b1"], np.float32)
    b2 = np.asarray(inputs["b2"], np.float32)

    attsd = np.zeros((64, 16), np.float32)
    for h in range(8):
        attsd[h * 8:(h + 1) * 8, h] = as1[h]
        attsd[h * 8:(h + 1) * 8, 8 + h] = ad1[h]
    att2sd = np.stack([as2, ad2], axis=1)

    common = {
        "w1": W1.astype(BF),
        "w2": W2.astype(BF),
        "attsd": attsd.astype(BF),
        "att2sd": att2sd.astype(BF),
        "b1c": np.tile(b1[None, :], (128, 1)).astype(np.float32),
        "b2c": np.tile(b2[None, :], (128, 1)).astype(np.float32),
        "rconst": np.tile(np.arange(128, dtype=np.float32)[None, :],
                          (128, 1)).astype(BF),
        "ident": np.eye(128, dtype=np.float32).astype(BF),
        "ident32": np.eye(128, dtype=np.float32),
        "zeros": np.zeros((128, 8 * TROW), BF),
    }
    S = int(meta["total_slots"])
    maps = []
    for core in range(NCORES):
        idx = meta["idx16"][core]
        idx_in = np.tile(idx.reshape(S // 16, 16).T, (8, 1))
        drel_in = meta["dstrel"][core].reshape(S // 128, 128).T.astype(BF)
        m = dict(common)
        m["xT"] = np.ascontiguousarray(
            x[core * SHARD:(core + 1) * SHARD].T).astype(BF)
        m["idxs"] = np.ascontiguousarray(idx_in)
        m["drel"] = np.ascontiguousarray(drel_in)
        m["ccnt"] = np.ascontiguousarray(
            meta["counts"][core].reshape(1, W * NQ).astype(np.int32))
        m["seltd"] = meta["seltd"][core]
        maps.append(m)
    return maps


# ---------------------------------------------------------------- bass build
def _edge_phase(nc, tc, meta, tbl_qs, idx_s, drel_s, rconst_s, ident_s,
                adw, acc, maxK, layer, zeros_d, ccnt_s, cregs, seltd):
    kq, cellmax16 = meta["kq"], meta["cellmax16"]
    cell_off, win_off, K = meta["cell_off"], meta["win_off"], meta["K"]
    cellmin = meta["cellmin"]
    H = 8 if layer == 1 else 1
    VPW = 72 if layer == 1 else 41
    GW = 9 if layer == 1 else 41
    AOFF = AS1_OFF if layer == 1 else AS2_OFF
    with tc.tile_pool(name=f"eg{layer}", bufs=4) as gpool, \
         tc.tile_pool(name=f"ep{layer}", bufs=2) as pool, \
         tc.tile_pool(name=f"epa{layer}", bufs=2, space="PSUM") as psa, \
         tc.tile_pool(name=f"epd{layer}", bufs=2, space="PSUM") as psd:
        for w in range(W):
            kw = int(K[w])
            adww = adw[:].rearrange("p (w h) -> p w h", w=W)[:, w, 0:H] \
                if layer == 1 else adw[:, w:w + 1]
            g = gpool.tile([128, maxK, TROW], bf16, tag="gather")
            if cregs is not None:
                nc.gpsimd.reg_load(cregs, ccnt_s[0:1, w * NQ:(w + 1) * NQ])
            for q in range(NQ):
                nidx = int(kq[w, q]) * 128
                if nidx == 0:
                    continue
                off_blk = int(cell_off[w, q]) // 128
                slot0 = int(win_off[w]) + int(cell_off[w, q])
                cmn = int(cellmin[w, q])
                if cmn < nidx:
                    zb = cmn // 128
                    nc.sync.dma_start(
                        g[:, off_blk + zb:off_blk + nidx // 128, :],
                        zeros_d[:, 0:(nidx // 128 - zb) * TROW].rearrange(
                            "p (n t) -> p n t", t=TROW))
                nc.gpsimd.dma_gather(
                    out_ap=g[:, off_blk:off_blk + nidx // 128, :],
                    in_ap=tbl_qs[q][:],
                    idxs_ap=idx_s[:, slot0 // 16:(slot0 + nidx) // 16],
                    num_idxs=nidx,
                    num_idxs_reg=(cregs[q] if cregs is not None
                                  else int(cellmax16[w, q])),
                    elem_size=TROW,
                    single_packet=False,
                    queue_num=q)
            sel = pool.tile([128, maxK * 128], bf16, tag="sel")
            nc.vector.tensor_tensor(
                out=sel[:, 0:kw * 128].rearrange("p (k j) -> p k j", k=kw),
                in0=drel_s[:, int(win_off[w]) // 128:int(win_off[w]) // 128 + kw]
                    .rearrange("p (k x) -> p k x", x=1).to_broadcast([128, kw, 128]),
                in1=rconst_s[:].rearrange("p (x j) -> p x j", x=1)
                    .to_broadcast([128, kw, 128]),
                op=ALU.is_equal)
            selt = pool.tile([128, maxK * 128], bf16, tag="selt")
            nc.sync.dma_start(
                selt[:, 0:kw * 128],
                seltd[:, int(win_off[w]):int(win_off[w]) + kw * 128])
            pad_ = psd.tile([128, maxK * H], f32, tag="adst")
            for k in range(kw):
                nc.tensor.matmul(
                    pad_[:, k * H:(k + 1) * H],
                    lhsT=selt[:, k * 128:(k + 1) * 128],
                    rhs=adww, start=True, stop=True)
            ep = pool.tile([128, maxK * H], f32, tag="ep")
            nc.vector.tensor_tensor(
                out=ep[:, 0:kw * H].rearrange("p (k h) -> p k h", k=kw),
                in0=g[:, 0:kw, AOFF:AOFF + H],
                in1=pad_[:, 0:kw * H].rearrange("p (k h) -> p k h", k=kw),
                op=ALU.add)
            pex1 = pool.tile([128, maxK * H], bf16, tag="pex1")
            nc.scalar.activation(pex1[:, 0:kw * H], ep[:, 0:kw * H], AF.Exp)
            pex = pool.tile([128, maxK * H], bf16, tag="pex")
            nc.scalar.activation(pex[:, 0:kw * H], ep[:, 0:kw * H], AF.Exp,
                                 scale=0.2)
            nc.vector.tensor_tensor(out=pex[:, 0:kw * H],
                                    in0=pex[:, 0:kw * H],
                                    in1=pex1[:, 0:kw * H], op=ALU.max)
            vp = pool.tile([128, maxK, VPW], bf16, tag="vp")
            nc.vector.tensor_tensor(
                out=vp[:, 0:kw, :].rearrange("p k (h x) -> p k h x", h=H),
                in0=g[:, 0:kw, 0:VPW].rearrange("p k (h x) -> p k h x", h=H),
                in1=pex[:, 0:kw * H].rearrange("p (k h x) -> p k h x", k=kw, x=1)
                    .to_broadcast([128, kw, H, GW]),
                op=ALU.mult)
            pagg = psa.tile([128, VPW], f32, tag="agg")
            for k in range(kw):
                nc.tensor.matmul(pagg[:], lhsT=sel[:, k * 128:(k + 1) * 128],
                                 rhs=vp[:, k, :], start=(k == 0),
                                 stop=(k == kw - 1))
            nc.scalar.activation(acc[:, w * VPW:(w + 1) * VPW], pagg[:],
                                 AF.Copy)


def _build(meta):
    kq = meta["kq"]; cellmax16 = meta["cellmax16"]
    K = meta["K"]; S = int(meta["total_slots"])
    SB = S // 128
    maxK = int(K.max())
    KT = FIN // 128

    nc = bacc.Bacc("TRN2", target_bir_lowering=False, debug=False,
                   num_devices=NCORES, num_swdge_queues=4)
    xT = nc.dram_tensor("xT", [FIN, SHARD], bf16, kind="ExternalInput")
    w1 = nc.dram_tensor("w1", [FIN, 64], bf16, kind="ExternalInput")
    w2 = nc.dram_tensor("w2", [64, NCLS], bf16, kind="ExternalInput")
    attsd = nc.dram_tensor("attsd", [64, 16], bf16, kind="ExternalInput")
    att2sd = nc.dram_tensor("att2sd", [NCLS, 2], bf16, kind="ExternalInput")
    b1c = nc.dram_tensor("b1c", [128, 64], f32, kind="ExternalInput")
    b2c = nc.dram_tensor("b2c", [128, NCLS], f32, kind="ExternalInput")
    rconst = nc.dram_tensor("rconst", [128, 128], bf16, kind="ExternalInput")
    ident = nc.dram_tensor("ident", [128, 128], bf16, kind="ExternalInput")
    ident32 = nc.dram_tensor("ident32", [128, 128], f32, kind="ExternalInput")
    idxs = nc.dram_tensor("idxs", [128, S // 16], i16, kind="ExternalInput")
    drel = nc.dram_tensor("drel", [128, SB], bf16, kind="ExternalInput")
    seltd = nc.dram_tensor("seltd", [128, S], bf16, kind="ExternalInput")
    zeros = nc.dram_tensor("zeros", [128, 8 * TROW], bf16, kind="ExternalInput")
    ccnt = nc.dram_tensor("ccnt", [1, W * NQ], i32, kind="ExternalInput")
    out = nc.dram_tensor("out", [SHARD_PAD, NCLS], f32, kind="ExternalOutput")

    with tile.TileContext(nc) as tc:
        with tc.tile_pool(name="dram", bufs=1, space="DRAM") as dpool, \
             tc.tile_pool(name="persist", bufs=1) as pp:
            tbl_shard = dpool.tile([SHARD_PAD, TROW], bf16)
            tbl_qs = [dpool.tile([QROWS, TROW], bf16, name=f"tbl_q{q}")
                      for q in range(NQ)]
            tbl_qs2 = [dpool.tile([QROWS, TROW], bf16, name=f"tbl2_q{q}")
                       for q in range(NQ)]

            ident_s = pp.tile([128, 128], bf16)
            nc.sync.dma_start(ident_s[:], ident[:])
            ident32_s = pp.tile([128, 128], f32)
            nc.sync.dma_start(ident32_s[:], ident32[:])
            rconst_s = pp.tile([128, 128], bf16)
            nc.sync.dma_start(rconst_s[:], rconst[:])
            drel_s = pp.tile([128, SB], bf16)
            nc.sync.dma_start(drel_s[:], drel[:])
            idx_s = pp.tile([128, S // 16], i16)
            nc.sync.dma_start(idx_s[:], idxs[:])
            b1_s = pp.tile([128, 64], f32)
            nc.sync.dma_start(b1_s[:], b1c[:])
            b2_s = pp.tile([128, NCLS], f32)
            nc.sync.dma_start(b2_s[:], b2c[:])
            ccnt_s = pp.tile([1, W * NQ], i32)
            nc.sync.dma_start(ccnt_s[:], ccnt[:])
            cregs = None  # register num_idxs_reg: suspected runtime crash
            acc = pp.tile([128, W * 72], f32, tag="acc")
            adw = pp.tile([128, W * 8], bf16, tag="adw")
            asw = pp.tile([128, W * 8], bf16, tag="asw")
            hloc = pp.tile([128, W * 72], bf16, tag="hloc")
            h2 = pp.tile([128, W * 64], f32, tag="h2")
            h2t = pp.tile([64, SHARD_PAD], bf16, tag="h2t")

            # phase A
            with tc.tile_pool(name="pa", bufs=2) as pool, \
                 tc.tile_pool(name="pac", bufs=1) as cpool, \
                 tc.tile_pool(name="pap", bufs=2, space="PSUM") as psum:
                w1_s = cpool.tile([128, KT, 64], bf16)
                nc.sync.dma_start(w1_s[:], w1[:].rearrange("(k p) m -> p k m", p=128))
                att_s = cpool.tile([64, 16], bf16)
                nc.sync.dma_start(att_s[:], attsd[:])
                hT = cpool.tile([64, SHARD], bf16)
                aT = cpool.tile([16, SHARD], bf16)
                stgbufs = [cpool.tile([128, TROW], bf16, name=f"stgA{i}")
                           for i in range(2)]
                for s_ in stgbufs:
                    nc.vector.memset(s_[:], 0.0)
                    nc.vector.memset(
                        s_[:, 0:72].rearrange("p (h c) -> p h c", h=8)[:, :, 8],
                        1.0)

                def emit_stg(w):
                    stg = stgbufs[w % 2]
                    n0 = w * 128
                    nw = min(128, SHARD - n0)
                    pt = psum.tile([128, 64], bf16, tag="tp")
                    nc.tensor.transpose(pt[0:nw, 0:64], hT[:, n0:n0 + nw],
                                        ident_s[0:64, 0:64])
                    nc.scalar.activation(
                        stg[0:nw, 0:72].rearrange("p (h c) -> p h c", h=8)[:, :, 0:8],
                        pt[0:nw, 0:64].rearrange("p (h c) -> p h c", h=8),
                        AF.Copy)
                    pt2 = psum.tile([128, 16], bf16, tag="tp2")
                    nc.tensor.transpose(pt2[0:nw, :], aT[:, n0:n0 + nw],
                                        ident_s[0:16, 0:16])
                    nc.scalar.activation(stg[0:nw, AS1_OFF:AS1_OFF + 16],
                                         pt2[0:nw, :], AF.Copy)
                    nc.sync.dma_start(
                        tbl_shard[:].rearrange("(w p) t -> p w t", p=128)[:, w, :],
                        stg[:])

                CH = 500
                next_w = 0
                for ci in range(SHARD // CH):
                    s0, s1 = ci * CH, (ci + 1) * CH
                    xt_t = pool.tile([128, KT, CH], bf16, tag="xt")
                    nc.sync.dma_start(
                        xt_t[:], xT[:, s0:s1].rearrange("(k p) n -> p k n", p=128))
                    ps = psum.tile([64, CH], f32, tag="hps")
                    for k in range(KT):
                        nc.tensor.matmul(ps[:], lhsT=w1_s[:, k, :],
                                         rhs=xt_t[:, k, :],
                                         start=(k == 0), stop=(k == KT - 1))
                    nc.vector.tensor_copy(hT[:, s0:s1], ps[:])
                    ps2 = psum.tile([16, CH], f32, tag="aps")
                    nc.tensor.matmul(ps2[:], lhsT=att_s[:], rhs=hT[:, s0:s1],
                                     start=True, stop=True)
                    nc.vector.tensor_copy(aT[:, s0:s1], ps2[:])
                    # emit table rows for fully-computed windows right away so
                    # the chunked AllGathers start while hT is still going
                    while next_w < W and (next_w + 1) * 128 <= s1:
                        emit_stg(next_w)
                        next_w += 1
                while next_w < W:
                    emit_stg(next_w)
                    next_w += 1

            for q in range(NQ):
                nc.gpsimd.collective_compute(
                    "AllGather", ALU.bypass,
                    replica_groups=[list(range(NCORES))],
                    ins=[tbl_shard[q * CHR:(q + 1) * CHR, :].opt()],
                    outs=[tbl_qs[q][:].opt()])
            nc.sync.dma_start(
                adw[:].rearrange("p (w h) -> p w h", w=W),
                tbl_shard[:].rearrange("(w p) t -> p w t", p=128)
                [:, :, AD1_OFF:AD1_OFF + 8])
            nc.sync.dma_start(
                asw[:].rearrange("p (w h) -> p w h", w=W),
                tbl_shard[:].rearrange("(w p) t -> p w t", p=128)
                [:, :, AS1_OFF:AS1_OFF + 8])
            nc.sync.dma_start(
                hloc[:].rearrange("p (w c) -> p w c", w=W),
                tbl_shard[:].rearrange("(w p) t -> p w t", p=128)[:, :, 0:72])

            _edge_phase(nc, tc, meta, tbl_qs, idx_s, drel_s, rconst_s,
                        ident_s, adw, acc, maxK, layer=1,
                        zeros_d=zeros, ccnt_s=ccnt_s, cregs=cregs,
                        seltd=seltd)

            # self-loop contribution, computed locally (never gathered)
            with tc.tile_pool(name="sl1", bufs=1) as pool:
                z = pool.tile([128, W * 8], f32)
                nc.vector.tensor_tensor(out=z[:], in0=asw[:], in1=adw[:],
                                        op=ALU.add)
                e1 = pool.tile([128, W * 8], bf16)
                nc.scalar.activation(e1[:], z[:], AF.Exp)
                pexs = pool.tile([128, W * 8], bf16)
                nc.scalar.activation(pexs[:], z[:], AF.Exp, scale=0.2)
                nc.vector.tensor_tensor(out=pexs[:], in0=pexs[:], in1=e1[:],
                                        op=ALU.max)
                vps = pool.tile([128, W * 72], f32)
                nc.vector.tensor_tensor(
                    out=vps[:].rearrange("p (w h x) -> p w h x", w=W, h=8),
                    in0=hloc[:].rearrange("p (w h x) -> p w h x", w=W, h=8),
                    in1=pexs[:].rearrange("p (w h x) -> p w h x", w=W, x=1)
                        .to_broadcast([128, W, 8, 9]),
                    op=ALU.mult)
                nc.vector.tensor_tensor(out=acc[:], in0=acc[:], in1=vps[:],
                                        op=ALU.add)

            with tc.tile_pool(name="f1", bufs=1) as pool:
                accv = acc[:].rearrange("p (w h x) -> p w h x", w=W, h=8)
                den = pool.tile([128, W * 8], f32)
                dv = den[:].rearrange("p (w h) -> p w h", w=W)
                nc.vector.tensor_scalar_add(dv, accv[:, :, :, 8], EPS)
                nc.vector.reciprocal(den[:], den[:])
                h2v = h2[:].rearrange("p (w h x) -> p w h x", w=W, h=8)
                nc.vector.tensor_tensor(
                    out=h2v, in0=accv[:, :, :, 0:8],
                    in1=den[:].rearrange("p (w h x) -> p w h x", w=W, x=1)
                        .to_broadcast([128, W, 8, 8]),
                    op=ALU.mult)
                nc.vector.tensor_tensor(
                    out=h2[:].rearrange("p (w x) -> p w x", w=W),
                    in0=h2[:].rearrange("p (w x) -> p w x", w=W),
                    in1=b1_s[:].rearrange("p (o x) -> p o x", o=1)
                        .to_broadcast([128, W, 64]),
                    op=ALU.add)
                t2 = pool.tile([128, W * 64], f32)
                nc.vector.tensor_scalar_min(t2[:], h2[:], 0.0)
                nc.scalar.activation(t2[:], t2[:], AF.Exp)
                nc.vector.tensor_scalar_add(t2[:], t2[:], -1.0)
                nc.vector.tensor_scalar_min(t2[:], t2[:], 0.0)
                nc.vector.tensor_scalar_max(h2[:], h2[:], 0.0)
                nc.vector.tensor_tensor(out=h2[:], in0=h2[:], in1=t2[:],
                                        op=ALU.add)

            # phase D
            with tc.tile_pool(name="pd", bufs=2) as pool, \
                 tc.tile_pool(name="pdc", bufs=1) as cpool, \
                 tc.tile_pool(name="pdp", bufs=1, space="PSUM") as psum:
                w2_s = cpool.tile([64, NCLS], bf16)
                nc.sync.dma_start(w2_s[:], w2[:])
                att2_s = cpool.tile([NCLS, 2], bf16)
                nc.sync.dma_start(att2_s[:], att2sd[:])
                for w in range(W):
                    pt = psum.tile([64, 128], f32, tag="t")
                    nc.tensor.transpose(pt[:], h2[:, w * 64:(w + 1) * 64],
                                        ident32_s[:])
                    nc.scalar.activation(h2t[:, w * 128:(w + 1) * 128], pt[:],
                                         AF.Copy)
                gT = cpool.tile([NCLS, SHARD_PAD], bf16)
                a2T = cpool.tile([2, SHARD_PAD], bf16)
                CH2 = 512
                for ci in range((SHARD_PAD + CH2 - 1) // CH2):
                    s0 = ci * CH2
                    s1 = min(s0 + CH2, SHARD_PAD)
                    ps = psum.tile([NCLS, CH2], f32, tag="g")
                    nc.tensor.matmul(ps[:, 0:s1 - s0], lhsT=w2_s[:],
                                     rhs=h2t[:, s0:s1], start=True, stop=True)
                    nc.vector.tensor_copy(gT[:, s0:s1], ps[:, 0:s1 - s0])
                    ps2 = psum.tile([2, CH2], f32, tag="a2")
                    nc.tensor.matmul(ps2[:, 0:s1 - s0], lhsT=att2_s[:],
                                     rhs=gT[:, s0:s1], start=True, stop=True)
                    nc.vector.tensor_copy(a2T[:, s0:s1], ps2[:, 0:s1 - s0])
                stgbufs2 = [cpool.tile([128, TROW], bf16, name=f"stgD{i}")
                            for i in range(2)]
                for s_ in stgbufs2:
                    nc.vector.memset(s_[:], 0.0)
                    nc.vector.memset(s_[:, NCLS:NCLS + 1], 1.0)
                for w in range(W):
                    stg = stgbufs2[w % 2]
                    pt = psum.tile([128, 64], bf16, tag="tg")
                    nc.tensor.transpose(pt[:, 0:NCLS],
                                        gT[:, w * 128:(w + 1) * 128],
                                        ident_s[0:NCLS, 0:NCLS])
                    nc.scalar.activation(stg[:, 0:NCLS], pt[:, 0:NCLS], AF.Copy)
                    pt2 = psum.tile([128, 2], bf16, tag="ta2")
                    nc.tensor.transpose(pt2[:], a2T[:, w * 128:(w + 1) * 128],
                                        ident_s[0:2, 0:2])
                    nc.scalar.activation(
                        stg[:, AS2_OFF:AD2_OFF + 8].rearrange(
                            "p (a x) -> p a x", a=2)[:, :, 0],
                        pt2[:], AF.Copy)
                    nc.sync.dma_start(
                        tbl_shard[:].rearrange("(w p) t -> p w t", p=128)[:, w, :],
                        stg[:])

            for q in range(NQ):
                nc.gpsimd.collective_compute(
                    "AllGather", ALU.bypass,
                    replica_groups=[list(range(NCORES))],
                    ins=[tbl_shard[q * CHR:(q + 1) * CHR, :].opt()],
                    outs=[tbl_qs2[q][:].opt()])
            nc.sync.dma_start(
                adw[:, 0:W].rearrange("p (w h) -> p w h", w=W),
                tbl_shard[:].rearrange("(w p) t -> p w t", p=128)
                [:, :, AD2_OFF:AD2_OFF + 1])
            nc.sync.dma_start(
                asw[:, 0:W].rearrange("p (w h) -> p w h", w=W),
                tbl_shard[:].rearrange("(w p) t -> p w t", p=128)
                [:, :, AS2_OFF:AS2_OFF + 1])
            nc.sync.dma_start(
                hloc[:, 0:W * 41].rearrange("p (w c) -> p w c", w=W),
                tbl_shard[:].rearrange("(w p) t -> p w t", p=128)[:, :, 0:41])

            acc2 = pp.tile([128, W * 41], f32, tag="acc")
            _edge_phase(nc, tc, meta, tbl_qs2, idx_s, drel_s, rconst_s,
                        ident_s, adw, acc2, maxK, layer=2,
                        zeros_d=zeros, ccnt_s=ccnt_s, cregs=cregs,
                        seltd=seltd)

            with tc.tile_pool(name="f2", bufs=2) as pool:
                for w0, w1 in ((0, W // 2), (W // 2, W)):
                    nw = w1 - w0
                    z = pool.tile([128, W - W // 2], f32, tag="z2")
                    nc.vector.tensor_tensor(out=z[:, 0:nw], in0=asw[:, w0:w1],
                                            in1=adw[:, w0:w1], op=ALU.add)
                    e1 = pool.tile([128, W - W // 2], bf16, tag="e12")
                    nc.scalar.activation(e1[:, 0:nw], z[:, 0:nw], AF.Exp)
                    pexs = pool.tile([128, W - W // 2], bf16, tag="pexs2")
                    nc.scalar.activation(pexs[:, 0:nw], z[:, 0:nw], AF.Exp,
                                         scale=0.2)
                    nc.vector.tensor_tensor(out=pexs[:, 0:nw],
                                            in0=pexs[:, 0:nw],
                                            in1=e1[:, 0:nw], op=ALU.max)
                    acc2s = acc2[:, w0 * 41:w1 * 41]
                    vps = pool.tile([128, (W - W // 2) * 41], f32, tag="vps2")
                    nc.vector.tensor_tensor(
                        out=vps[:, 0:nw * 41]
                            .rearrange("p (w x) -> p w x", w=nw),
                        in0=hloc[:, w0 * 41:w1 * 41]
                            .rearrange("p (w x) -> p w x", w=nw),
                        in1=pexs[:, 0:nw].rearrange("p (w x) -> p w x", x=1)
                            .to_broadcast([128, nw, 41]),
                        op=ALU.mult)
                    nc.vector.tensor_tensor(out=acc2s, in0=acc2s,
                                            in1=vps[:, 0:nw * 41], op=ALU.add)
                    accv = acc2s.rearrange("p (w x) -> p w x", w=nw)
                    den = pool.tile([128, W - W // 2], f32, tag="den2")
                    nc.vector.tensor_scalar_add(den[:, 0:nw], accv[:, :, 40],
                                                EPS)
                    nc.vector.reciprocal(den[:, 0:nw], den[:, 0:nw])
                    o = pool.tile([128, (W - W // 2) * NCLS], f32, tag="o2")
                    ov = o[:, 0:nw * NCLS].rearrange("p (w x) -> p w x", w=nw)
                    nc.vector.tensor_tensor(
                        out=ov, in0=accv[:, :, 0:NCLS],
                        in1=den[:, 0:nw].rearrange("p (w x) -> p w x", x=1)
                            .to_broadcast([128, nw, NCLS]),
                        op=ALU.mult)
                    nc.vector.tensor_tensor(
                        out=ov, in0=ov,
                        in1=b2_s[:].rearrange("p (o x) -> p o x", o=1)
                            .to_broadcast([128, nw, NCLS]),
                        op=ALU.add)
                    mx = pool.tile([128, W - W // 2], f32, tag="mx2")
                    nc.vector.tensor_reduce(out=mx[:, 0:nw], in_=ov,
                                            op=ALU.max,
                                            axis=mybir.AxisListType.X)
                    nc.vector.tensor_tensor(
                        out=ov, in0=ov,
                        in1=mx[:, 0:nw].rearrange("p (w x) -> p w x", x=1)
                            .to_broadcast([128, nw, NCLS]),
                        op=ALU.subtract)
                    nc.scalar.activation(o[:, 0:nw * NCLS],
                                         o[:, 0:nw * NCLS], AF.Exp)
                    sm = pool.tile([128, W - W // 2], f32, tag="sm2")
                    nc.vector.tensor_reduce(out=sm[:, 0:nw], in_=ov,
                                            op=ALU.add,
                                            axis=mybir.AxisListType.X)
                    nc.vector.reciprocal(sm[:, 0:nw], sm[:, 0:nw])
                    nc.vector.tensor_tensor(
                        out=ov, in0=ov,
                        in1=sm[:, 0:nw].rearrange("p (w x) -> p w x", x=1)
                            .to_broadcast([128, nw, NCLS]),
                        op=ALU.mult)
                    nc.sync.dma_start(
                        out[:].rearrange("(w p) x -> p w x", p=128)[:, w0:w1],
                        ov)
    nc.finalize()
    return nc


# ---------------------------------------------------------------- entry point
def kernel(**inputs):
    edge = np.asarray(inputs["edge_index"])
    key = hash(edge[:, :1024].tobytes()) ^ hash(edge.shape)
    if key not in _CACHE:
        meta = _prep(edge)
        nc = _build(meta)
        _CACHE[key] = (meta, nc)
    meta, nc = _CACHE[key]
    maps = _build_inputs(meta, inputs)
    res = bass_utils.run_bass_kernel_spmd(
        nc, maps, core_ids=list(range(NCORES)), trace=False)
    out = np.zeros((N, NCLS), np.float32)
    for core in range(NCORES):
        o = np.asarray(res.results[core]["out"]).reshape(SHARD_PAD, NCLS)
        out[core * SHARD:(core + 1) * SHARD] = o[:SHARD]
    return out

